# revision 1
# baseline (speedup 1.0000x reference)
"""GNN message-passing layer (normalized-adjacency conv + linear + LeakyReLU)
on 8 Trainium2 NeuronCores, pure data parallel over the batch dim.

Computation (per batch b):
    deg      = adj.sum(-1)                     # [N]
    agg      = (adj / deg[:, None]) @ X        # [N, FIN]
    out      = leakyrelu(agg @ W.T + bias)     # [N, FOUT]

Device-side formulation. adj is host-transposed per batch (adjT[k, m] =
adj[m, k]) so the contraction index k sits on SBUF partitions for both matmul
operands, and everything downstream stays transposed ([feature, node] order)
so all PE work streams 512-wide:
    rawT[f, m]   = sum_k X[k, f] * adjT[k, m]    # X tiles as weights, fp32r
    degbc[:, m]  = sum_k 1 * adjT[k, m]          # ones[128,128] weights ->
                                                 # deg broadcast to all parts
    out2T[o, m]  = sum_f WT[f, o] * rawT[f, m]   # W as weights, fp32r
    t            = out2T / degbc                 # DVE divide
    outT[o, m]   = alpha*(t + b) + (1-alpha)*Relu(t + b)   # b is per-partition
The DRAM output is [B, FOUT, N]; the host swaps the last two axes.

The matmuls run in fp32r (fp32 with 11 explicit mantissa bits; 1 PE cycle/row
instead of 4): adjT/x/wT are pre-rounded to fp32r on the host
(round-to-nearest-even on the dropped 12 bits) and declared float32r
end-to-end; rawT is rounded to fp32r by the PSUM->SBUF copy. deg multiplies
the rounded values by exactly-representable 1.0, so deg is exact w.r.t. the
rounded adjacency; bias stays exact fp32.
"""

import numpy as np

import concourse.bass as bass
import concourse.mybir as mybir
import concourse.tile as tile
from concourse.bass_utils import run_bass_kernel_spmd

P = 128

# Problem shape (hardcoded per the harness contract).
B, N, FIN, FOUT = 32, 1024, 128, 128
NEG_SLOPE = 0.01
N_CORES = 8
BPC = B // N_CORES  # batches per core


def build_bass(nbatch=BPC, n=N, fin=FIN, fout=FOUT, neg_slope=NEG_SLOPE,
               adj_bufs=5, use_f32r=True, f32r_second=True):
    f32 = mybir.dt.float32
    mmdt = mybir.dt.float32r if use_f32r else f32
    rdt = mybir.dt.float32r if (use_f32r and f32r_second) else f32
    alpha = float(neg_slope)
    nc = bass.Bass()

    adjT = nc.dram_tensor("adjT", [nbatch, n, n], mmdt, kind="ExternalInput")
    x = nc.dram_tensor("x", [nbatch, P, n // P, fin], mmdt,
                       kind="ExternalInput")
    onesW = nc.dram_tensor("onesW", [P, P], mmdt, kind="ExternalInput")
    wT = nc.dram_tensor("wT", [fin, fout], rdt, kind="ExternalInput")
    bvec = nc.dram_tensor("bvec", [P, 1], f32, kind="ExternalInput")
    outT = nc.dram_tensor("outT", [nbatch, fout, n], f32, kind="ExternalOutput")

    KT = n // P          # contraction tiles
    CH = min(512, n)     # matmul moving free dim (one fp32 PSUM bank)
    NCH = n // CH        # moving-dim chunks

    with tile.TileContext(nc) as tc:
        with (
            tc.tile_pool(name="const", bufs=1) as cpool,
            tc.tile_pool(name="adj", bufs=adj_bufs) as apool,
            tc.tile_pool(name="xt", bufs=2) as xpool,
            tc.tile_pool(name="raw", bufs=2) as rpool,
            tc.tile_pool(name="post", bufs=4) as opool,
            tc.tile_pool(name="psr", bufs=3, space="PSUM") as ps_raw,
            tc.tile_pool(name="psd", bufs=2, space="PSUM") as ps_deg,
            tc.tile_pool(name="pso", bufs=2, space="PSUM") as ps_out,
        ):
            wT_sb = cpool.tile([fin, fout], rdt, tag="w")
            nc.sync.dma_start(wT_sb[:], wT[:, :])
            b_sb = cpool.tile([P, 1], f32, tag="b")
            nc.sync.dma_start(b_sb[:], bvec[:, :])
            # (1-alpha)*b for the fused Relu bias
            b2_sb = cpool.tile([P, 1], f32, tag="b2")
            nc.vector.tensor_scalar_mul(b2_sb[:], b_sb[:], 1.0 - alpha)
            onesW_sb = cpool.tile([P, P], mmdt, tag="onesW")
            nc.sync.dma_start(onesW_sb[:], onesW[:, :])

            for b in range(nbatch):
                x_sb = xpool.tile([P, KT, fin], mmdt, tag="x")
                nc.sync.dma_start(x_sb[:], x[b])

                # adj in two 2 MB dma_starts (>=1 MiB per transfer for full
                # SDMA fan-out), each carrying KG k-tiles
                KG = KT // 2
                adj_chunks = []
                for c2 in range(2):
                    ac = apool.tile([P, KG, n], mmdt, tag="adj", name=f"ac{c2}")
                    nc.sync.dma_start(
                        ac[:],
                        adjT[b, c2 * KG * P:(c2 + 1) * KG * P, :]
                        .rearrange("(g p) m -> p g m", p=P),
                    )
                    adj_chunks.append(ac)

                def adj_slice(k, c):
                    return adj_chunks[k // KG][:, k % KG, c * CH:(c + 1) * CH]

                # rawT matmuls, one accumulation group per 512-chunk
                ps_chunks = [
                    ps_raw.tile([P, CH], f32, tag="psraw", name=f"psraw{cc}")
                    for cc in range(NCH)
                ]
                for k in range(KT):
                    for c in range(NCH):
                        nc.tensor.matmul(
                            ps_chunks[c][:, :],
                            x_sb[:, k, :],
                            adj_slice(k, c),
                            start=(k == 0),
                            stop=(k == KT - 1),
                        )

                # Partial k-tile sums for deg on the DVE (tree, 7 adds);
                # the ones-weights matmul below folds the remaining 128
                # partitions and broadcasts deg to every output partition.
                def aslc(k):
                    return adj_chunks[k // KG][:, k % KG, :]

                half = KT // 2
                acc_a = rpool.tile([P, n], mmdt, tag="acca")
                nc.vector.tensor_tensor(
                    acc_a[:, :], aslc(0), aslc(1), mybir.AluOpType.add)
                for k in range(2, half):
                    nc.vector.tensor_tensor(
                        acc_a[:, :], acc_a[:, :], aslc(k), mybir.AluOpType.add)
                acc = rpool.tile([P, n], mmdt, tag="accc")
                if KT > 2:
                    acc_b = rpool.tile([P, n], mmdt, tag="accb")
                    nc.vector.tensor_tensor(
                        acc_b[:, :], aslc(half), aslc(half + 1),
                        mybir.AluOpType.add)
                    for k in range(half + 2, KT):
                        nc.vector.tensor_tensor(
                            acc_b[:, :], acc_b[:, :], aslc(k),
                            mybir.AluOpType.add)
                    nc.vector.tensor_tensor(
                        acc[:, :], acc_a[:, :], acc_b[:, :], mybir.AluOpType.add)
                else:
                    nc.vector.tensor_copy(acc[:, :], acc_a[:, :])

                raw_sb = rpool.tile([P, n], rdt, tag="raw")
                for c in range(NCH):
                    nc.scalar.copy(raw_sb[:, c * CH:(c + 1) * CH], ps_chunks[c][:, :])

                o_full = opool.tile([P, n], f32, tag="ofull")
                for c in range(NCH):
                    # deg broadcast to all partitions via ones weights
                    ps_db = ps_deg.tile([P, CH], f32, tag="psdeg")
                    nc.tensor.matmul(
                        ps_db[:, :],
                        onesW_sb[:, :],
                        acc[:, c * CH:(c + 1) * CH],
                        start=True,
                        stop=True,
                    )
                    # 1/deg on the scalar engine (reciprocal LUT; its error is
                    # quadratically suppressed nowhere here, so the HW rel-err
                    # check guards it). bass refuses Reciprocal directly, so
                    # emit a Copy and flip the func.
                    rec_sb = opool.tile([P, CH], f32, tag="rec")
                    _ai = nc.scalar.activation(
                        rec_sb[:, :], ps_db[:, :],
                        mybir.ActivationFunctionType.Copy, bias=0.0, scale=1.0)
                    _ai.ins.func = mybir.ActivationFunctionType.Reciprocal

                    # out2T[o, m] = sum_f WT[f, o] * rawT[f, m]
                    ps_o = ps_out.tile([P, CH], f32, tag="psout")
                    nc.tensor.matmul(
                        ps_o[:, :],
                        wT_sb[:, :],
                        raw_sb[:, c * CH:(c + 1) * CH],
                        start=True,
                        stop=True,
                    )
                    # t = out2T / deg
                    t_sb = opool.tile([P, CH], f32, tag="t")
                    nc.vector.tensor_tensor(
                        t_sb[:, :], ps_o[:, :], rec_sb[:, :],
                        mybir.AluOpType.mult,
                    )
                    # u = alpha * (t + b)
                    u_sb = opool.tile([P, CH], f32, tag="u")
                    nc.vector.tensor_scalar(
                        u_sb[:, :], t_sb[:, :], b_sb[:, 0:1], alpha,
                        mybir.AluOpType.add, mybir.AluOpType.mult,
                    )
                    # r = Relu((1-alpha)*t + (1-alpha)*b) = (1-alpha)*Relu(t+b)
                    r_sb = opool.tile([P, CH], f32, tag="r")
                    nc.scalar.activation(
                        r_sb[:, :], t_sb[:, :],
                        mybir.ActivationFunctionType.Relu,
                        bias=b2_sb[:, 0:1], scale=1.0 - alpha,
                    )
                    # outT = u + r = leaky(t + b)
                    nc.vector.tensor_tensor(
                        o_full[:, c * CH:(c + 1) * CH], u_sb[:, :], r_sb[:, :],
                        mybir.AluOpType.add,
                    )
                nc.sync.dma_start(outT[b], o_full[:, :])

    _split_multi_waits(nc)
    return nc


def _split_multi_waits(nc):
    """Walrus rejects split-struct instructions (fp32/fp32r fused-weight-load
    matmult, TensorScalarPtr, ...) with more than one sync wait ("Too many
    sync wait commands" in setupSyncWait<...>). Hoist all but the last wait
    of each multi-wait instruction onto same-engine no-ops inserted
    immediately before it (one wait per no-op)."""
    cnt = 0
    for f in nc.m.functions:
        for blk in f.blocks:
            idx = 0
            while idx < len(blk.instructions):
                inst = blk.instructions[idx]
                si = inst.sync_info
                if (type(inst).__name__ != "InstNoOp" and si is not None
                        and len(si.on_wait) > 1):
                    waits = list(si.on_wait)
                    for w in waits[:-1]:
                        nop = mybir.InstNoOp(name=f"mm_wait_nop_{cnt}",
                                             ins=[], outs=[])
                        cnt += 1
                        nop.engine = inst.engine
                        nop.sync_info = mybir.SyncInfo(on_wait=[w],
                                                       on_update=[])
                        nc.register_instruction(nop)
                        blk.instructions.insert(idx, nop)
                        idx += 1
                    inst.sync_info = mybir.SyncInfo(
                        on_wait=waits[-1:], on_update=list(si.on_update))
                idx += 1
    return cnt


_NC_CACHE = {}

USE_F32R = True
F32R_SECOND = True


def _get_nc():
    if "nc" not in _NC_CACHE:
        _NC_CACHE["nc"] = build_bass(use_f32r=USE_F32R, f32r_second=F32R_SECOND)
    return _NC_CACHE["nc"]


def _round_fp32r(a):
    """Round fp32 values to fp32r (11 explicit mantissa bits), RNE."""
    u = np.ascontiguousarray(a, dtype=np.float32).view(np.uint32)
    r = (u + np.uint32(0x7FF) + ((u >> np.uint32(12)) & np.uint32(1))) \
        & np.uint32(0xFFFFF000)
    return r.view(np.float32)


def _prep_in_maps(node_mat, adj_mat, W, b):
    node_mat = np.ascontiguousarray(node_mat, dtype=np.float32)
    adj_mat = np.asarray(adj_mat, dtype=np.float32)
    wT = np.ascontiguousarray(np.asarray(W, dtype=np.float32).T)
    if USE_F32R and F32R_SECOND:
        wT = _round_fp32r(wT)
    bvec = np.ascontiguousarray(
        np.asarray(b, dtype=np.float32).reshape(P, 1))
    onesW = np.ones((P, P), dtype=np.float32)
    in_maps = []
    for c in range(N_CORES):
        sl = slice(c * BPC, (c + 1) * BPC)
        adjT = np.ascontiguousarray(adj_mat[sl].transpose(0, 2, 1))
        xs = np.ascontiguousarray(
            node_mat[sl].reshape(BPC, N // P, P, FIN).transpose(0, 2, 1, 3))
        if USE_F32R:
            adjT = _round_fp32r(adjT)
            xs = _round_fp32r(xs)
        in_maps.append({
            "adjT": adjT,
            "x": xs,
            "onesW": onesW,
            "wT": wT,
            "bvec": bvec,
        })
    return in_maps


def kernel(node_mat, adj_mat, W, b):
    nc = _get_nc()
    in_maps = _prep_in_maps(node_mat, adj_mat, W, b)
    res = run_bass_kernel_spmd(nc, in_maps, core_ids=list(range(N_CORES)))
    return np.ascontiguousarray(
        np.concatenate(
            [res.results[c]["outT"] for c in range(N_CORES)], axis=0
        ).swapaxes(1, 2)
    )



# revision 2
# speedup vs baseline: 2.0080x; 2.0080x over previous
"""GNN message-passing layer (normalized-adjacency conv + linear + LeakyReLU)
on 8 Trainium2 NeuronCores, pure data parallel over the batch dim.

Computation (per batch b):
    deg = adj.sum(-1); out = leakyrelu((adj/deg) @ X @ W.T + bias)

The kernel is HBM-bandwidth-bound (adj is 4 MB/batch at fp32), so everything
is cast to bf16 on the host and the 1/deg row-scaling is folded into adj
host-side (norm_adj = adj/deg, the exact expression the reference computes,
then one RNE round to bf16).  Per-core traffic drops 21 MB -> 10.3 MB and
bf16 matmuls stream at 1 PE cycle/row (fp32r measured ~2 on HW).

Device-side, per batch (all tensors transposed so the contraction index sits
on SBUF partitions and PE work streams 512-wide):
    XW[p, o]   = sum_f Xt_g[f, p] * wT[f, o]      per k-tile g (8 matmuls)
    out2T[o,m] = sum_k XW_k[p, o] * normadjT[k, m]  (16 matmuls, PSUM accum)
    outT[o, m] = Lrelu(out2T + bias)              one fused ACT op per chunk
The DVE does only the XW PSUM->SBUF bf16 copy; there is no deg/reciprocal
work on device at all.  DRAM output is [B, FOUT, N] bf16; the host upcasts
and swaps the last two axes.

Host pre-swizzles adj/x so every DMA descriptor is contiguous per partition
(adj: two 1 MB chunks per batch, 8 KB/partition runs; x/out: 2 KB runs).
"""

import numpy as np
import ml_dtypes

import concourse.bass as bass
import concourse.mybir as mybir
import concourse.tile as tile
from concourse.bass_utils import run_bass_kernel_spmd

P = 128

# Problem shape (hardcoded per the harness contract).
B, N, FIN, FOUT = 32, 1024, 128, 128
NEG_SLOPE = 0.01
N_CORES = 8
BPC = B // N_CORES  # batches per core

KT = N // P      # 8 contraction k-tiles
NHALF = 2        # adj DMA chunks per batch
HG = KT // NHALF  # k-tiles per adj chunk
CH = 512         # matmul moving free dim (one fp32 PSUM bank)
NCH = N // CH


def build_bass(nbatch=BPC, n=N, fin=FIN, fout=FOUT, neg_slope=NEG_SLOPE):
    f32 = mybir.dt.float32
    bf16 = mybir.dt.bfloat16
    nc = bass.Bass()

    # adj[b, h, p, g, m] = normadjT[b, (h*HG+g)*P + p, m]
    adj = nc.dram_tensor("adj", [nbatch, NHALF, P, HG, n], bf16,
                         kind="ExternalInput")
    # x[b, f, g, p] = node[b, g*P + p, f]   (per-k-tile transposed X)
    x = nc.dram_tensor("x", [nbatch, P, KT, P], bf16, kind="ExternalInput")
    wT = nc.dram_tensor("wT", [fin, fout], bf16, kind="ExternalInput")
    bvec = nc.dram_tensor("bvec", [P, 1], f32, kind="ExternalInput")
    outT = nc.dram_tensor("outT", [nbatch, fout, n], bf16,
                          kind="ExternalOutput")

    with tile.TileContext(nc) as tc:
        with (
            tc.tile_pool(name="const", bufs=1) as cpool,
            tc.tile_pool(name="adj", bufs=6) as apool,
            tc.tile_pool(name="xt", bufs=3) as xpool,
            tc.tile_pool(name="xw", bufs=2) as xwpool,
            tc.tile_pool(name="out", bufs=3) as opool,
            tc.tile_pool(name="psxw", bufs=1, space="PSUM") as ps_xw,
            tc.tile_pool(name="psm", bufs=4, space="PSUM") as ps_main,
        ):
            wT_sb = cpool.tile([fin, fout], bf16, tag="w")
            nc.sync.dma_start(wT_sb[:], wT[:, :])
            b_sb = cpool.tile([P, 1], f32, tag="b")
            nc.sync.dma_start(b_sb[:], bvec[:, :])

            for b in range(nbatch):
                x_sb = xpool.tile([P, KT, P], bf16, tag="x")
                nc.sync.dma_start(x_sb[:], x[b])

                adj_chunks = []
                for h in range(NHALF):
                    ac = apool.tile([P, HG, n], bf16, tag="adj", name=f"ac{h}")
                    nc.sync.dma_start(ac[:], adj[b, h])
                    adj_chunks.append(ac)

                # XW^T tiles: XW[p, o] = sum_f X[gP+p, f] W[o, f]
                ps_xwt = ps_xw.tile([P, KT * P], f32, tag="psxw")
                for g in range(KT):
                    nc.tensor.matmul(
                        ps_xwt[:, g * P:(g + 1) * P],
                        x_sb[:, g, :],
                        wT_sb[:, :],
                        start=True,
                        stop=True,
                    )
                xw_sb = xwpool.tile([P, KT * P], bf16, tag="xw")
                nc.vector.tensor_copy(xw_sb[:, :], ps_xwt[:, :])

                # out2T[o, m] accumulated over k-tiles, one group per chunk
                ps_c = [
                    ps_main.tile([P, CH], f32, tag="psm", name=f"psm{c}")
                    for c in range(NCH)
                ]
                for h in range(NHALF):
                    for g in range(HG):
                        k = h * HG + g
                        for c in range(NCH):
                            nc.tensor.matmul(
                                ps_c[c][:, :],
                                xw_sb[:, k * P:(k + 1) * P],
                                adj_chunks[h][:, g, c * CH:(c + 1) * CH],
                                start=(k == 0),
                                stop=(k == KT - 1),
                            )

                o_sb = opool.tile([P, n], bf16, tag="o")
                for c in range(NCH):
                    nc.scalar.activation(
                        o_sb[:, c * CH:(c + 1) * CH],
                        ps_c[c][:, :],
                        mybir.ActivationFunctionType.Lrelu,
                        bias=b_sb[:, 0:1],
                        scale=1.0,
                        alpha=float(neg_slope),
                    )
                nc.sync.dma_start(outT[b], o_sb[:, :])

    _split_multi_waits(nc)
    return nc


def _split_multi_waits(nc):
    """Walrus rejects split-struct instructions (fp32/fp32r fused-weight-load
    matmult, TensorScalarPtr, ...) with more than one sync wait ("Too many
    sync wait commands" in setupSyncWait<...>). Hoist all but the last wait
    of each multi-wait instruction onto same-engine no-ops inserted
    immediately before it (one wait per no-op)."""
    cnt = 0
    for f in nc.m.functions:
        for blk in f.blocks:
            idx = 0
            while idx < len(blk.instructions):
                inst = blk.instructions[idx]
                si = inst.sync_info
                if (type(inst).__name__ != "InstNoOp" and si is not None
                        and len(si.on_wait) > 1):
                    waits = list(si.on_wait)
                    for w in waits[:-1]:
                        nop = mybir.InstNoOp(name=f"mm_wait_nop_{cnt}",
                                             ins=[], outs=[])
                        cnt += 1
                        nop.engine = inst.engine
                        nop.sync_info = mybir.SyncInfo(on_wait=[w],
                                                       on_update=[])
                        nc.register_instruction(nop)
                        blk.instructions.insert(idx, nop)
                        idx += 1
                    inst.sync_info = mybir.SyncInfo(
                        on_wait=waits[-1:], on_update=list(si.on_update))
                idx += 1
    return cnt


_NC_CACHE = {}


def _get_nc():
    if "nc" not in _NC_CACHE:
        _NC_CACHE["nc"] = build_bass()
    return _NC_CACHE["nc"]


def _prep_in_maps(node_mat, adj_mat, W, b):
    bf16 = ml_dtypes.bfloat16
    node_mat = np.ascontiguousarray(node_mat, dtype=np.float32)
    adj_mat = np.asarray(adj_mat, dtype=np.float32)
    # Fold the degree normalization into adj (same fp32 expression as the
    # reference), then one RNE round to bf16.
    norm = adj_mat / adj_mat.sum(axis=-1, keepdims=True)
    wT = np.ascontiguousarray(np.asarray(W, dtype=np.float32).T).astype(bf16)
    bvec = np.ascontiguousarray(
        np.asarray(b, dtype=np.float32).reshape(P, 1))
    in_maps = []
    for c in range(N_CORES):
        sl = slice(c * BPC, (c + 1) * BPC)
        # adjT[k, m] -> [h, p, g, m] with k = (h*HG + g)*P + p
        adjT = norm[sl].transpose(0, 2, 1)
        adj_sw = np.ascontiguousarray(
            adjT.reshape(BPC, NHALF, HG, P, N).transpose(0, 1, 3, 2, 4)
        ).astype(bf16)
        # x[f, g, p] = node[g*P + p, f]
        x_sw = np.ascontiguousarray(
            node_mat[sl].reshape(BPC, KT, P, FIN).transpose(0, 3, 1, 2)
        ).astype(bf16)
        in_maps.append({
            "adj": adj_sw,
            "x": x_sw,
            "wT": wT,
            "bvec": bvec,
        })
    return in_maps


def kernel(node_mat, adj_mat, W, b):
    nc = _get_nc()
    in_maps = _prep_in_maps(node_mat, adj_mat, W, b)
    res = run_bass_kernel_spmd(nc, in_maps, core_ids=list(range(N_CORES)))
    return np.ascontiguousarray(
        np.concatenate(
            [res.results[c]["outT"].astype(np.float32) for c in range(N_CORES)],
            axis=0,
        ).swapaxes(1, 2)
    )
